# revision 4
# baseline (speedup 1.0000x reference)
"""STCN/STM-style memory read (retrieval_knn) on 8 Trainium2 NeuronCores.

v2: mixed-precision readout. Reference computation per batch b:
    mk  [64, 8000]  memory keys     (THW = 5*40*40 = 8000)
    mv  [512, 8000] memory values
    qk  [64, 1600]  query keys      (HW = 1600)
    sim = (2 * mk.T @ qk - ||mk||^2) / 8          # [8000, 1600]
    attn = softmax(sim, axis=0)
    out = mv @ attn                                # [512, 1600]

Sharding: 8 cores = 4 batches x 2 query-halves (800 query pixels/core).
Memory axis padded 8000 -> 8064 = 63 tiles of 128.

The kernel is Tensor-engine bound (readout matmuls = 63 x 4cv x 800q
cols at 1 col/cycle fp16). v2 moves the last 20 m-tiles (10 pairs) of
the value readout to fp8e4m3 DoubleRow matmuls: 2 m-tiles contracted
per matmul at 0.5 cycles/col = 4x throughput. Value quantization error
is cancelled with an fp8 hi+lo split (2 accumulating matmuls); the
softmax-weight fp8 quantization (~2.5% rms) only touches 20/63 of the
softmax mass => ~1.5e-2 output rel err (gate: 2e-2). A +C8 bias inside
the exp keeps the fp8 tiles' weights inside e4m3's normal range; it
cancels between numerator and denominator.

On-chip dataflow per core (q in 2 chunks of 400, m-tiles in pairs):
    sim_psum[128m, 2*400] = 2x matmul(lhsT=mk_aug[128, 128m], rhs=qk_aug)
    exp[128,2,400]        = ScalarE Exp(0.25*psum + C8)  (fp16, or fp8
                            for the 10 DoubleRow pairs)
    out_psum[128cv, 400] += matmul(mv16_tile, exp half)      (fp16 tiles)
    out_psum[128cv, 400] += DR-matmul(mv8hi/lo pair, exp8)   (fp8 pairs)
    acc16[128, 400]      += exp halves       (VectorE, fp16 2x mode)
    den = ones^T @ acc; recip = 1/den; bcast; out_sb = psum * bcast
    DMA out fp16 -> host upcasts to fp32.
"""

import sys

sys.path.insert(0, "/opt/trn_rl_repo")

import numpy as np
import ml_dtypes

B, CK, CV, T, H, W = 4, 64, 512, 5, 40, 40
THW = T * H * W          # 8000
HW = H * W               # 1600
NT = 63                  # number of 128-row memory tiles after padding
MPAD = NT * 128          # 8064
KDIM = 128               # padded contraction dim (64 keys + 2 aug + zeros)
NCORES = 8
Q = HW // 2              # 800 query pixels per core
CHUNKS = (400, 400)      # per-core query chunk sizes (sum = Q)
NCV = CV // 128          # 4
PAD_MKSQ = 4.0e4         # ||mk||^2 for padded memory rows -> sim=-5000 -> exp=0
C8 = float(np.log(120.0) - 3.72)   # exp bias: top weight ~120 < e4m3 max 240

N8PAIRS = 10             # trailing m-tile pairs computed in fp8 DoubleRow
FP8_T0 = 62 - 2 * N8PAIRS  # first fp8 tile (42): tiles 42..61 fp8, 62 fp16

F8NP = ml_dtypes.float8_e4m3

_CACHE = {}
LAST_RESULTS = None      # BassKernelResults of the most recent run (for test.py)


def _build_program(n_reps=1, chunks=CHUNKS, ebufs=4):
    import concourse.bacc as bacc
    import concourse.bass as bass
    import concourse.mybir as mybir
    import concourse.tile as tile
    from concourse.bass import ts

    f8 = mybir.dt.float8e4
    f16 = mybir.dt.float16
    f32 = mybir.dt.float32
    Exp = mybir.ActivationFunctionType.Exp
    DR = mybir.MatmulPerfMode.DoubleRow

    nc = bacc.Bacc(None, target_bir_lowering=False)

    mk_d = nc.dram_tensor("mk", [KDIM, MPAD], f16, kind="ExternalInput")
    qk_d = nc.dram_tensor("qk", [KDIM, Q], f16, kind="ExternalInput")
    # mv partition-major: mvt[p, t*CV + c] = value[c, m = t*128 + p]
    # so a run of tiles is one contiguous 2D slice = one DMA
    mv_d = nc.dram_tensor("mvt", [128, 43 * CV], f16, kind="ExternalInput")
    mv8h_d = nc.dram_tensor("mv8h", [128, N8PAIRS * 2 * CV], f8, kind="ExternalInput")
    mv8l_d = nc.dram_tensor("mv8l", [128, N8PAIRS * 2 * CV], f8, kind="ExternalInput")
    out_d = nc.dram_tensor("out", [CV, Q], f16, kind="ExternalOutput")

    with tile.TileContext(nc) as tc:
        with (
            tc.tile_pool(name="const", bufs=1) as cpool,
            tc.tile_pool(name="keys", bufs=1) as kpool,
            tc.tile_pool(name="mv", bufs=1) as mvpool,
            tc.tile_pool(name="work", bufs=2) as wpool,
            tc.tile_pool(name="exps", bufs=ebufs) as epool,
            tc.tile_pool(name="osb", bufs=4) as opool,
            tc.tile_pool(name="ps_out", bufs=4, space="PSUM") as ps_out,
            tc.tile_pool(name="ps_sim", bufs=2, space="PSUM") as ps_sim,
        ):
            ones_col = cpool.tile([128, 1], f16, name="ones_col")
            nc.vector.memset(ones_col[:], 1.0)
            ones_row = cpool.tile([1, 128], f32, name="ones_row")
            nc.vector.memset(ones_row[:], 1.0)
            c8_bias = cpool.tile([128, 1], f32, name="c8_bias")
            nc.vector.memset(c8_bias[:], C8)

            import contextlib

            loop_ctx = (
                tc.For_i(0, n_reps, 1, hint_engines=(mybir.EngineType.PE,))
                if n_reps > 1
                else contextlib.nullcontext()
            )
            with loop_ctx:
                r = "r0_"
                # DMA issue order == consumption order; the SP sequencer
                # takes ~565ns per dma_start, so issues are few and graded
                # (small first pieces unblock the pipeline fastest).
                qk_s = kpool.tile([KDIM, Q], f16, name=r + "qk_s", tag="qk")
                mk_pieces = [(62, 1), (0, 2), (2, 2), (4, 4), (8, 6), (14, 8),
                             (42, 8), (50, 8), (58, 4), (22, 7), (29, 7),
                             (36, 6)]
                mk_tiles_sb = {}

                def issue_mk(j):
                    t0, ntile = mk_pieces[j]
                    sz = ntile * 128
                    mkp = kpool.tile(
                        [KDIM, sz], f16, name=f"{r}mk_s{j}", tag=f"mk{j}"
                    )
                    nc.sync.dma_start(mkp[:], mk_d[:, bass.ds(t0 * 128, sz)])
                    for i in range(ntile):
                        mk_tiles_sb[t0 + i] = mkp[:, ts(i, 128)]

                def mk_tile(t):
                    return mk_tiles_sb[t]

                # fp16 value tile groups; slot order in mv_d is the
                # consumption order: [62, 0..41]
                mv_groups = [1, 2, 2, 4, 6, 8, 7, 7, 6]   # tiles per group
                mv_slot_tiles = [NT - 1] + list(range(FP8_T0))
                mv_tiles = {}
                mv_state = {"slot": 0}

                def issue_mv(g, eng=None):
                    gsz = mv_groups[g]
                    s0 = mv_state["slot"]
                    grp = mvpool.tile(
                        [128, gsz * CV], f16, name=f"{r}mvg{g}", tag=f"mvg{g}"
                    )
                    (eng or nc.sync).dma_start(
                        grp[:], mv_d[:, bass.ds(s0 * CV, gsz * CV)]
                    )
                    for i in range(gsz):
                        mv_tiles[mv_slot_tiles[s0 + i]] = grp[
                            :, bass.ds(i * CV, CV)
                        ]
                    mv_state["slot"] = s0 + gsz

                mv8 = [None] * N8PAIRS
                mv8_groups = [(0, 4), (4, 3), (7, 3)]      # (first pair, n)

                def issue_mv8(g):
                    j0, gsz = mv8_groups[g]
                    gh = mvpool.tile(
                        [128, gsz, 2, CV], f8, name=f"{r}mv8h{g}", tag=f"mv8h{g}"
                    )
                    gl = mvpool.tile(
                        [128, gsz, 2, CV], f8, name=f"{r}mv8l{g}", tag=f"mv8l{g}"
                    )
                    csl = bass.ds(j0 * 2 * CV, gsz * 2 * CV)
                    nc.sync.dma_start(gh[:], mv8h_d[:, csl])
                    nc.sync.dma_start(gl[:], mv8l_d[:, csl])
                    for i in range(gsz):
                        mv8[j0 + i] = (gh[:, i], gl[:, i])

                # qk chunk 1 goes out on the Activation DGE concurrently
                # with the SP-issued mk/mv stream; everything else follows
                # in consumption order on SP.
                nc.scalar.dma_start(qk_s[:, : chunks[0]], qk_d[:, : chunks[0]])
                issue_mk(0)          # mk tile 62        (SP #1)
                issue_mv(0, nc.scalar)  # mv tile 62     (ACT #2, in parallel)
                issue_mk(1)          # mk tiles 0-1      (SP #2)
                issue_mv(1)          # mv tiles 0-1
                qo = chunks[0]
                for csz in chunks[1:]:
                    nc.sync.dma_start(
                        qk_s[:, qo : qo + csz], qk_d[:, qo : qo + csz]
                    )
                    qo += csz
                issue_mk(2); issue_mv(2)
                issue_mk(3); issue_mv(3)
                issue_mk(4); issue_mv(4)
                issue_mk(5); issue_mv(5)          # fp16 tiles through 21
                issue_mk(6); issue_mv8(0)         # fp8 region
                issue_mk(7); issue_mv8(1)
                issue_mk(8); issue_mv8(2)
                issue_mk(9); issue_mv(6)          # fp16 tiles 22..41
                issue_mk(10); issue_mv(7)
                issue_mk(11); issue_mv(8)

                # m-tile schedule: tile 62 (fp16, half padding) first,
                # fp8 DoubleRow pairs in the middle (their mv8 DMAs arrive
                # later than the first fp16 groups), fp16 pairs last so the
                # final drain is a plain fp16 pipeline.
                pairs = (
                    [(NT - 1,)]
                    + [(p, p + 1) for p in range(0, 22, 2)]
                    + [(p, p + 1) for p in range(FP8_T0, NT - 1, 2)]
                    + [(p, p + 1) for p in range(22, FP8_T0, 2)]
                )
                last_pi = len(pairs) - 1

                # m-tiles processed in pairs: one [128, 2, 512] PSUM tile
                # holds sim for (t, t+1); a single ScalarE Exp covers
                # both; PE computes the next pair's sims meanwhile.
                def make_sim_pair(qc, qsl, CSZ, pr):
                    simp = ps_sim.tile(
                        [128, len(pr), 512], f32, name=f"{r}sim{qc}_{pr[0]}", tag="sim"
                    )
                    for i, t in enumerate(pr):
                        nc.tensor.matmul(
                            simp[:, i, :CSZ],
                            mk_tile(t),
                            qk_s[:, qsl],
                            start=True,
                            stop=True,
                        )
                    return simp

                qoffs = []
                qoff = 0
                for CSZ in chunks:
                    qoffs.append(qoff)
                    qoff += CSZ

                premade = {}
                for qc, CSZ in enumerate(chunks):
                    qsl = bass.ds(qoffs[qc], CSZ)
                    last = qc == len(chunks) - 1
                    if not last:
                        nqsl = bass.ds(qoffs[qc + 1], chunks[qc + 1])
                        ncsz = chunks[qc + 1]
                    outs_ps = [
                        ps_out.tile([128, CSZ], f32, name=f"{r}o{qc}_{cv}", tag="out")
                        for cv in range(NCV)
                    ]
                    acc = wpool.tile([128, CSZ], f16, name=f"{r}acc{qc}", tag="acc")

                    carry = None
                    if 0 not in premade:
                        premade[0] = make_sim_pair(qc, qsl, CSZ, pairs[0])
                    next_premade = {}
                    for pi, pr in enumerate(pairs):
                        cur = premade.pop(pi, carry)
                        is8 = len(pr) == 2 and pr[0] >= FP8_T0
                        expp = epool.tile(
                            [128, len(pr), CSZ],
                            f8 if is8 else f16,
                            name=f"{r}e{qc}_{pr[0]}",
                            tag="exp",
                        )
                        nc.scalar.activation(
                            expp[:], cur[:, :, :CSZ], Exp, bias=c8_bias[:], scale=0.25
                        )
                        if pi + 1 <= last_pi:
                            if pi + 1 in premade:
                                carry = None
                            else:
                                carry = make_sim_pair(qc, qsl, CSZ, pairs[pi + 1])
                        elif not last:
                            # prefetch next chunk's first sim pair so the PE
                            # queue has work ahead of this chunk's den/bc
                            # matmuls (which wait on DVE acc / reciprocal)
                            next_premade[0] = make_sim_pair(
                                qc + 1, nqsl, ncsz, pairs[0]
                            )
                        if is8:
                            j = (pr[0] - FP8_T0) // 2
                            h, l = mv8[j]
                            for cv in range(NCV):
                                for mvp in (h, l):
                                    nc.tensor.matmul(
                                        outs_ps[cv][:],
                                        mvp[:, :, ts(cv, 128)],
                                        expp[:, :, :],
                                        start=False,
                                        stop=False,
                                        perf_mode=DR,
                                    )
                        for i, t in enumerate(pr):
                            esl = expp[:, i, :]
                            if pi == 0:
                                nc.vector.tensor_copy(acc[:], esl)
                            else:
                                nc.vector.tensor_add(acc[:], acc[:], esl)
                            if not is8:
                                for cv in range(NCV):
                                    nc.tensor.matmul(
                                        outs_ps[cv][:],
                                        mv_tiles[t][:, ts(cv, 128)],
                                        esl,
                                        start=(pi == 0),
                                        stop=(pi == last_pi and i == len(pr) - 1),
                                    )

                    den = ps_sim.tile([1, CSZ], f32, name=f"{r}den{qc}", tag="sim")
                    nc.tensor.matmul(
                        den[:], ones_col[:], acc[:], start=True, stop=True
                    )
                    if not last:
                        # a second prefetched pair lands between den and bc,
                        # covering the reciprocal latency on the PE queue
                        next_premade[1] = make_sim_pair(
                            qc + 1, nqsl, ncsz, pairs[1]
                        )
                    recip = wpool.tile([1, CSZ], f32, name=f"{r}rcp{qc}", tag="rcp")
                    nc.vector.reciprocal(recip[:], den[:])
                    bc = ps_sim.tile([128, CSZ], f32, name=f"{r}bc{qc}", tag="sim")
                    nc.tensor.matmul(
                        bc[:], ones_row[:], recip[:], start=True, stop=True
                    )
                    bc_sb = wpool.tile([128, CSZ], f32, name=f"{r}bcs{qc}", tag="bcs")
                    nc.scalar.copy(bc_sb[:], bc[:])
                    for cv in range(NCV):
                        o_sb = opool.tile(
                            [128, CSZ], f16, name=f"{r}os{qc}_{cv}", tag="osb"
                        )
                        nc.vector.tensor_mul(o_sb[:], outs_ps[cv][:], bc_sb[:])
                        # alternate the output-DMA issue between the SP and
                        # Activation DGEs so the drain isn't serialized on
                        # one sequencer
                        eng = nc.scalar if cv % 2 else nc.sync
                        eng.dma_start(out_d[ts(cv, 128), qsl], o_sb[:])
                    premade = next_premade

    nc.compile()
    return nc


def _get_program():
    if "nc" not in _CACHE:
        _CACHE["nc"] = _build_program()
    return _CACHE["nc"]


def host_prep(mem_key, mem_val, qry_key):
    """Layout/sharding prep: returns per-core input maps."""
    mem_key = np.asarray(mem_key, dtype=np.float32)
    mem_val = np.asarray(mem_val, dtype=np.float32)
    qry_key = np.asarray(qry_key, dtype=np.float32)

    mk = mem_key.reshape(B, CK, THW)
    mksq = np.einsum("bcm,bcm->bm", mk, mk)                    # [B, THW]

    # mk_aug rows: 0:64 keys, 64 = ||mk||^2 (fp16 hi), 65 = residual (lo),
    # 66:128 zero.  Padded memory columns get mksq=4e4 -> softmax weight 0.
    mk16 = np.zeros((B, KDIM, MPAD), np.float16)
    mk16[:, :CK, :THW] = mk
    mk16[:, CK, :] = PAD_MKSQ
    hi = mksq.astype(np.float16)
    mk16[:, CK, :THW] = hi
    mk16[:, CK + 1, :THW] = (mksq - hi.astype(np.float32)).astype(np.float16)

    qk16 = np.zeros((B, KDIM, HW), np.float16)
    qk16[:, :CK] = qry_key.reshape(B, CK, HW)
    qk16[:, CK : CK + 2] = -0.5

    mvt = np.zeros((B, MPAD, CV), np.float32)
    mvt[:, :THW, :] = mem_val.reshape(B, CV, THW).transpose(0, 2, 1)

    # fp16 tiles packed partition-major in consumption order [62, 0..41]:
    # mv16[b][p, s*CV + c] = mvt[b, tile_s*128 + p, c]
    slots = [NT - 1] + list(range(FP8_T0))
    mv16 = np.empty((B, 128, len(slots) * CV), np.float16)
    for s, t in enumerate(slots):
        mv16[:, :, s * CV : (s + 1) * CV] = mvt[:, t * 128 : (t + 1) * 128, :]

    # fp8 hi/lo pair blocks for tiles FP8_T0 .. FP8_T0 + 2*N8PAIRS - 1
    # layout [B, 128, pair, slot, c]: slot = pair member (DoubleRow k-tile)
    mv8h = np.zeros((B, 128, N8PAIRS, 2, CV), F8NP)
    mv8l = np.zeros((B, 128, N8PAIRS, 2, CV), F8NP)
    for j in range(N8PAIRS):
        for i in range(2):
            t = FP8_T0 + 2 * j + i
            blk = mvt[:, t * 128 : (t + 1) * 128, :]           # [B,128,CV]
            h = blk.astype(F8NP)
            l = (blk - h.astype(np.float32)).astype(F8NP)
            mv8h[:, :, j, i] = h
            mv8l[:, :, j, i] = l
    mv8h = mv8h.reshape(B, 128, N8PAIRS * 2 * CV)
    mv8l = mv8l.reshape(B, 128, N8PAIRS * 2 * CV)

    in_maps = []
    for c in range(NCORES):
        b, h_ = divmod(c, 2)
        in_maps.append(
            {
                "mk": mk16[b],
                "qk": np.ascontiguousarray(qk16[b, :, h_ * Q : (h_ + 1) * Q]),
                "mvt": mv16[b],
                "mv8h": mv8h[b],
                "mv8l": mv8l[b],
            }
        )
    return in_maps


def kernel(mem_key, mem_val, qry_key):
    global LAST_RESULTS
    import os

    # this container's axon client has no NTFF hook; the trace path would
    # crash run_bass_kernel_spmd, so force it off
    os.environ["BASS_NEVER_TRACE"] = "1"
    from concourse.bass_utils import run_bass_kernel_spmd

    in_maps = host_prep(mem_key, mem_val, qry_key)
    nc = _get_program()
    LAST_RESULTS = run_bass_kernel_spmd(nc, in_maps, list(range(NCORES)))

    out = np.empty((B, CV, HW), np.float32)
    for c in range(NCORES):
        b, h = divmod(c, 2)
        out[b, :, h * Q : (h + 1) * Q] = LAST_RESULTS.results[c]["out"].astype(
            np.float32
        )
    return out.reshape(B, CV, H, W)
